# revision 45
# baseline (speedup 1.0000x reference)
"""AdaptiveNeuromorphicNetwork Trainium2 kernel (8 NeuronCores, SPMD).

Sharding: output neurons H=2048 split 256/core (H-shard) -> the LIF scan,
spike-rate mean (over batch) and homeostatic threshold update are fully local
per core; zero collectives. input_spikes are replicated (each core streams all
of them through the TensorEngine against its weight column shard).

Per-core pipeline over time-chunks (CHUNKS, small->large ramp so the scan
starts early, tiny last chunk to shrink the drain tail):
  fp8 spike DMA (host-relayouted partition-major: one contiguous multi-KB
  descriptor per partition) -> 3-plane fp8 DoubleRow matmul into a shared
  PSUM group -> PSUM->SBUF evacuation (scalar engine, x 1/wscale descale) ->
  segmented tensor_tensor_scan turns the per-step weighted sums into i_syn
  for the whole chunk in one DVE op -> per-step LIF on the DVE -> cc-scaled
  spikes accumulate in SBUF (bf16) -> chunked DMA out.

Matmul precision (fp8x4): W * wscale (wscale = 238/max|w|, plane-0 peak just
under the fp8-e4m3 max) is decomposed into NPLANES=3 residual fp8-e4m3
planes (each plane the fp8 rounding of the remaining residual). All planes
run DoubleRow (2 k-tiles per matmul at 0.5 cyc/row -> 0.25 cyc per
k-tile*column, 4x cheaper than fp16), sharing one PSUM accumulation at the
common wscale; a single activation-copy descales on evacuation. Spikes are
exactly 0/1 in fp8, so one fp8 spike DMA feeds all planes. Weight error rms
~1.8e-6 (fp8 subnormal floor) -> output rel err ~9.6e-3 (gate 2e-2).
2 planes measure rel err 0.055 (fail); bf16 or int16 formulations cost
4x/2x more PE cycles per contraction row -- fp8x3-DoubleRow is optimal
under the TimelineSim cost model (cols x 0.5cyc x 0.4167ns, 3 planes
= 86.2us PE busy, the compute floor of this kernel).

The LIF scan (the end-to-end critical path, ~585ns/step steady-state on the
DVE; DVE busy ~94us total -- the overall pacer together with the PE):
  i_syn: one tensor_tensor_scan per chunk computes the whole chunk's
    i_syn EMA (state = mask*state + w; a per-chunk-size mask holds a_syn
    with 0 at each (h,b) segment's t=0 column to reset the recurrence, and
    the first w column is pre-fixed to a_syn*carry + w0 by one STT).
  LIF_SC (custom DVE op, per h-tile): s' = cc*(((a_mem*v + i) + negThr)>=0),
    accum_out = sum_b s' = cc*rate. Output spikes are cc-scaled; the host
    recovers them as (out != 0).
  critical nT update: nT' = NT1 + cc*rs -- ONE DVE tensor_tensor issued
    right after the LIF_SC pair (no cross-engine hop on the serial chain;
    per-op DVE cost is 60ns fixed + 1.04ns/elem, so the [128,64] LIF ops
    are 127ns and the [128,2] TT is 63ns).
  LIF_V (custom DVE op): v' = P + s*negThr (recomputes s from the old nT).
  off the critical path (gpsimd): T1' = 0.99*(T1 + cc*rs) + k1 and
    NT1' = nT' + T1', where T1 = 0.99*R + k1 and NT1 = nT + T1 (algebraic
    unrolling of fre/threshold EMAs: R = lr*tgt - lr*fre).

Startup: all weight planes as one DMA then chunk-0 spikes, strictly ordered
on one queue (the DMA engines are modeled as a serial resource); chunk
sizes ramp 2,3,5,... so the first evacuation (which gates the scan) happens
as early as possible; warmup matmuls keep the PE p-state ramped while the
startup DMAs stream. Emission order is a scheduling HINT only: the Tile
framework's static scheduler reorders instructions per engine by
(bass_priority, dependency readiness), which is why seemingly-equivalent
emission layouts measure differently.
"""
import numpy as np

import concourse.bass as bass
import concourse.tile as tile
from concourse import bacc, mybir
from concourse.bass_utils import run_bass_kernel_spmd

B, I, H, T = 64, 2048, 2048, 128
NCORES = 8
HL = H // NCORES            # 256 output neurons per core
KT = I // 128               # 16 contraction tiles
CHUNKS = [2, 3, 5, 6, 7, 8, 9, 11, 12, 12, 12, 12, 12, 15, 2]   # per-chunk step counts
NCH = len(CHUNKS)
assert sum(CHUNKS) == T
DT = 0.001

NPLANES = 3         # fp8 residual planes
W_SCALE_NUM = 238.0  # plane-0 peak target: scale = 238/max|w| (fp8 max 240)
TRACE = False
TRACE_KW = {}
REPEAT = 1          # execute the whole pipeline N times (timing builds only)
WARMUP_MM = 55       # dummy matmuls bridging the startup DMAs (PE p-state)
BUF_WEV = 8         # evacuated-weighted-chunk buffers (scan run-behind depth)
BUF_SPK = 3         # spike-chunk prefetch buffers
BUF_IP = 2          # i_syn chunk buffers
BUF_ACC = 3         # output-accumulator buffers
BUF_TMP = 3         # small per-step scratch (rs/u) buffers
SCALAR_Q_CHUNKS = (1, 2)  # chunks whose spike DMA issues on the scalar queue
OUT_FP8 = False     # bf16 spike output (cc-scaled values; cc not fp8-representable)

_F32 = mybir.dt.float32
_ALU = mybir.AluOpType

# ---- custom fused DVE ops for the LIF step ----
import operator as _op

import concourse.dve_ops as _dve_ops
from concourse.dve_ops import DveOp as _DveOp
from concourse.dve_spec import (Spec as _Spec, Src0 as _Src0, Src1 as _Src1,
                                C0 as _C0, C1 as _C1, C2 as _C2, Zero as _Zero,
                                lower as _lower, _has_src1)
from concourse.dve_table_gen import dve_ver_for as _dve_ver_for
from concourse.dve_uop import DveOpSpec as _DveOpSpec


def _register_dve(name, spec):
    if name in _dve_ops._SUB_OPCODE_FOR_NAME:
        for o in _dve_ops.OPS:
            if o.name == name:
                return o
    ver = _dve_ver_for("TRN2")
    opcode = max(_dve_ops._SUB_OPCODE_FOR_NAME.values()) + 1
    assert opcode < 0x20
    sha = _DveOpSpec(name=name, opcode=opcode, uops=_lower(spec, ver=ver),
                     rd1_en=_has_src1(spec)).sha(ver)
    dop = _DveOp(name, spec, subdim=False, uops_sha={ver: sha})
    _dve_ops.OPS.append(dop)
    _dve_ops.CUSTOM_DVE_SPECS[name] = spec
    _dve_ops._SUB_OPCODE_FOR_NAME[name] = opcode
    return dop


_P = _Src1 * _C0 + _Src0


def _lif_sc_ref(in0, in1, s0, s1, imm2):
    P = in1.astype(np.float32) * s0 + in0
    s = (P + s1 >= 0).astype(np.float32) * imm2
    return s, s.reshape(s.shape[0], -1).sum(axis=-1, keepdims=True)


# s' = (((v*a_mem + isyn) + negThr) >= 0) * cc ; accum = cc*ns
# (output is cc-scaled spikes; host recovers spikes as (out != 0))
LIF_SC = _register_dve("LIF_SC",
                       _Spec(body=((_P + _C1) >= _Zero) * _C2,
                             accum=_op.add, reference=_lif_sc_ref))


def _lif_v_ref(in0, in1, s0, s1, imm2):
    P = in1.astype(np.float32) * s0 + in0
    s = (P + s1 >= 0).astype(np.float32)
    return P + s * s1


# v' = P + ((P + negThr) >= 0) * negThr
LIF_V = _register_dve("LIF_V", _Spec(body=_P + ((_P + _C1) >= _Zero) * _C1,
                                     reference=_lif_v_ref))


def _col_blocks(n, blk=512):
    """Split n columns into PSUM-bank-sized (<=512 f32) blocks."""
    return [(c, min(c + blk, n)) for c in range(0, n, blk)]


def _build_fp8x4(a_mem, a_syn, lr, tgt, wscale):
    """All-fp8 multi-plane DoubleRow pipeline (see module docstring)."""
    nc = bacc.Bacc("TRN2", target_bir_lowering=False, debug=False,
                   num_devices=NCORES)
    f8 = mybir.dt.float8e4
    NP = NPLANES
    # weights: [i128, (plane, kp, ht, ko, h)] -> per-plane contiguous DMAs
    wgt8 = nc.dram_tensor("wgt8", [128, NP * KT * 2 * 128], f8,
                          kind="ExternalInput").ap()
    # partition-major spike layout: [p, (chunk, k, b, t)]
    spk8 = nc.dram_tensor("spk8", [128, KT * B * T], f8,
                          kind="ExternalInput").ap()
    nt0 = nc.dram_tensor("nt0", [128, 2], _F32, kind="ExternalInput").ap()
    odt = f8 if OUT_FP8 else mybir.dt.bfloat16
    out = nc.dram_tensor("out", [128, T * 128], odt, kind="ExternalOutput").ap()

    a_mem, a_syn, lr, tgt = float(a_mem), float(a_syn), float(lr), float(tgt)
    c_ema = float(np.float32(-lr / 6400.0))
    k1 = float(np.float32(0.01 * lr * tgt))
    r0 = float(np.float32(lr * tgt))
    descale = 1.0 / float(wscale)
    PL = KT * 2 * 128           # per-plane weight columns
    _CP = mybir.ActivationFunctionType.Copy

    with tile.TileContext(nc) as tc:
        with tc.tile_pool(name="wpool", bufs=1) as wpool, \
             tc.tile_pool(name="state", bufs=1) as state, \
             tc.tile_pool(name="spkp", bufs=BUF_SPK) as spkp, \
             tc.tile_pool(name="psum", bufs=2, space="PSUM") as psum, \
             tc.tile_pool(name="wev", bufs=BUF_WEV) as wev, \
             tc.tile_pool(name="ipool", bufs=BUF_IP) as ipool, \
             tc.tile_pool(name="accp", bufs=BUF_ACC) as accp, \
             tc.tile_pool(name="tmp", bufs=BUF_TMP) as tmp:

            # ---- persistent tiles ----
            # Startup DMAs on the sync queue in service order: plane-0
            # weights (its matmuls start ~1.6us in), chunk-0 spikes, then
            # the later planes. The chunk loop's c>=1 spike DMAs queue
            # behind on the same queue.
            wsb8 = wpool.tile([128, NP * PL], f8, tag="wsb8")
            wsb8p = [wsb8[:, p * PL:(p + 1) * PL] for p in range(NP)]
            nc.sync.dma_start(wsb8[:], wgt8[:])
            spk_c0 = spkp.tile([128, KT * B * CHUNKS[0]], f8, tag="spk8",
                               name="spk8_c0")
            nc.sync.dma_start(spk_c0[:], spk8[:, 0:KT * B * CHUNKS[0]])
            # tiny threshold-state memsets FIRST on the pool (they gate the
            # first LIF steps); the big scan-mask fills go to the ACT engine
            # (idle until the first evacuation).
            # segment masks for the i_syn tensor_tensor_scan: a_syn
            # everywhere, 0 at each (h,b) segment's t=0 column (gpsimd:
            # keeps the DVE free for the scan)
            masks = {}
            for TCv in sorted(set(CHUNKS)):
                mk = state.tile([128, 128 * TCv], _F32, tag=f"mask{TCv}",
                                name=f"mask{TCv}")
                nc.gpsimd.memset(mk[:], a_syn)
                m3 = mk[:].rearrange("p (m t) -> p m t", t=TCv)
                nc.gpsimd.memset(m3[:, :, 0:1], 0.0)
                masks[TCv] = mk
            # negThr double-buffer: step t reads nTs[t%2], the critical
            # update writes nTs[(t+1)%2]
            nTs = [state.tile([128, 2], _F32, tag=f"nT{i}", name=f"nT{i}")
                   for i in range(2)]
            nc.scalar.dma_start(nTs[0][:], nt0[:])
            T1st = state.tile([128, 2], _F32, tag="T1st")
            nc.gpsimd.memset(T1st[:], float(np.float32(0.99 * r0 + k1)))
            NT1st = state.tile([128, 2], _F32, tag="NT1st")
            nc.gpsimd.tensor_tensor(NT1st[:], nTs[0][:], T1st[:], op=_ALU.add)
            K1t = state.tile([128, 2], _F32, tag="K1t")
            nc.vector.memset(K1t[:], k1)
            C99t = state.tile([128, 2], _F32, tag="C99t")
            nc.gpsimd.memset(C99t[:], 0.99)

            vst = [state.tile([128, 128], _F32, tag=f"v{i}", name=f"v{i}")
                   for i in range(2)]
            nc.vector.memset(vst[0][:], 0.0)
            # PE p-state warmup: dummy matmuls keep the PE busy while the
            # startup DMAs stream, so the real matmuls run at full clock.
            ps_c0 = [psum.tile([128, B * CHUNKS[0]], _F32, tag=f"ps{ht}",
                               name=f"ps_c0_{ht}") for ht in range(2)]
            warm = state.tile([128, 128], f8, tag="warm")
            nc.vector.memset(warm[:], 0.0)
            for _w in range(WARMUP_MM):
                nc.tensor.matmul(ps_c0[0][:, 0:min(128, B * CHUNKS[0])],
                                 warm[:], warm[:], start=(_w == 0),
                                 stop=(_w == WARMUP_MM - 1),
                                 skip_group_check=True)

            def emit_mm_evac(c, t0c, TC):
                """Spike DMA + multi-plane matmul + descaling evacuation."""
                BTC = B * TC
                cols0 = B * t0c
                if c == 0 and REPEAT == 1:
                    spk8_t = spk_c0
                else:
                    spk8_t = spkp.tile([128, KT * BTC], f8, tag="spk8",
                                       name=f"spk8_c{c}")
                    # chunks 1-2 go via the scalar queue: their descriptor
                    # generation overlaps the sync queue's
                    q = nc.scalar if c in SCALAR_Q_CHUNKS else nc.sync
                    q.dma_start(
                        spk8_t[:],
                        spk8[:, KT * cols0:KT * cols0 + KT * BTC])
                if c == 0 and REPEAT == 1:
                    ps = ps_c0
                else:
                    ps = [psum.tile([128, BTC], _F32, tag=f"ps{ht}",
                                    name=f"ps{c}_{ht}") for ht in range(2)]
                blocks = _col_blocks(BTC)
                for p in range(NP):
                    for kp in range(KT // 2):
                        for ht in range(2):
                            l8 = wsb8p[p][:, ((kp * 2 + ht) * 2) * 128:
                                          ((kp * 2 + ht) * 2 + 2) * 128
                                          ].rearrange("p (ko h) -> p ko h",
                                                      ko=2)
                            r8 = spk8_t[:, (2 * kp) * BTC:
                                        (2 * kp + 2) * BTC].rearrange(
                                "p (ko n) -> p ko n", ko=2)
                            for c0, c1 in blocks:
                                nc.tensor.matmul(
                                    ps[ht][:, c0:c1],
                                    l8, r8[:, :, c0:c1],
                                    start=(p == 0 and kp == 0),
                                    stop=(p == NP - 1
                                          and kp == KT // 2 - 1),
                                    perf_mode=mybir.MatmulPerfMode.DoubleRow)
                # wt_ev layout: [p, (h, b, t)]
                wt_ev = wev.tile([128, 2 * BTC], _F32, tag="wt_ev",
                                 name=f"wt_ev_c{c}")
                if c == 0:
                    # chunk 0 gates the whole scan: evacuate on the (idle)
                    # DVE itself -- no PE->ACT->DVE semaphore round-trip
                    for ht in range(2):
                        nc.vector.tensor_scalar_mul(
                            wt_ev[:, ht * BTC:(ht + 1) * BTC],
                            ps[ht][:], descale)
                else:
                    with tc.high_priority():
                        for ht in range(2):
                            nc.scalar.activation(
                                wt_ev[:, ht * BTC:(ht + 1) * BTC],
                                ps[ht][:],
                                mybir.ActivationFunctionType.Copy,
                                bias=0.0, scale=descale)
                return wt_ev

            def emit_carry_fix(TC, wt_ev, i_prev, TCp):
                """Pre-fix the first w column to a_syn*carry + w0."""
                wv = wt_ev[:].rearrange("p (m t) -> p m t", t=TC)
                pv = i_prev[:].rearrange("p (m t) -> p m t", t=TCp)
                nc.vector.scalar_tensor_tensor(
                    wv[:, :, 0:1], pv[:, :, TCp - 1:TCp], a_syn,
                    wv[:, :, 0:1], op0=_ALU.mult, op1=_ALU.add)

            def emit_tts(c, TC, wt_ev):
                """i_syn for a whole chunk: one segmented tensor_tensor_scan."""
                BTC = B * TC
                i_all = ipool.tile([128, 2 * BTC], _F32, tag="i_all",
                                   name=f"i_all_c{c}")
                nc.vector.tensor_tensor_scan(
                    i_all[:], masks[TC][:], wt_ev[:], 0.0,
                    op0=_ALU.mult, op1=_ALU.add)
                return i_all

            def emit_ttscan(c, TC, wt_ev, i_prev, TCp):
                if i_prev is not None:
                    emit_carry_fix(TC, wt_ev, i_prev, TCp)
                return emit_tts(c, TC, wt_ev)

            def emit_scan(c, TC, t0c, i_all, nxt):
                """Per-step LIF for chunk c. nxt = (c+1, TC+1, wt_ev+1) or
                None; when present, the next chunk's carry-fix + scan are
                issued just before this chunk's LAST step so their sem
                ack-latency hides behind scan work. Returns the next chunk's
                i_all (or None).

                nT' = NT1 + (cc/2)*rs is the only op on the step-to-step
                critical path (one DVE STT right after the LIF_SC pair).
                T1' = 0.99*(T1 + (cc/2)*rs) + k1 and NT1' = nT' + T1' are
                maintained on gpsimd, off the critical path."""
                i4 = i_all[:].rearrange("p (h b t) -> p h b t", h=2, b=B)
                i_next = None
                acc = accp.tile([128, TC * 128], odt, tag="acc",
                                name=f"acc_c{c}")
                for tl in range(TC):
                    inject = nxt is not None and tl == TC - 1
                    if inject:
                        # next chunk's carry-fix first; its SBUF write-ack
                        # hides behind this step's LIF_SC pair, and the
                        # scan's ack behind the LIF_V pair below
                        cn, TCn, wtn = nxt
                        emit_carry_fix(TCn, wtn, i_all, TC)
                    t = t0c + tl
                    last = (t == T - 1)
                    vold, vnew = vst[t % 2], vst[(t + 1) % 2]
                    nTo, nTn = nTs[t % 2], nTs[(t + 1) % 2]
                    rs = tmp.tile([128, 2], _F32, tag="rs", name=f"rs{t}")
                    for ht in range(2):
                        s_out = acc[:, tl * 128 + ht * B:
                                    tl * 128 + (ht + 1) * B]
                        # s' = cc*(((a_mem*v + i) + nT) >= 0) ; rs = cc*ns
                        nc.vector._custom_dve(
                            LIF_SC, out=s_out,
                            in0=i4[:, ht, :, tl],
                            in1=vold[:, ht * B:(ht + 1) * B], s0=a_mem,
                            s1=nTo[:, ht:ht + 1], imm2=c_ema,
                            accum_out=rs[:, ht:ht + 1])
                    if inject:
                        i_next = emit_tts(cn, TCn, wtn)
                    if not last:
                        # critical: nT' = NT1 + cc*rs
                        nc.vector.tensor_tensor(nTn[:], NT1st[:], rs[:],
                                                op=_ALU.add)
                        for ht in range(2):
                            sl = slice(ht * B, (ht + 1) * B)
                            # v' = P + s*nT (recomputes s from the old nT)
                            nc.vector._custom_dve(
                                LIF_V, out=vnew[:, sl],
                                in0=i4[:, ht, :, tl],
                                in1=vold[:, sl], s0=a_mem,
                                s1=nTo[:, ht:ht + 1])
                        # off-path threshold state (gpsimd):
                        # u = T1 + cc*rs ; u2 = 0.99*u ; T1' = u2 + k1 ;
                        # NT1' = nT' + T1'
                        u = tmp.tile([128, 2], _F32, tag="u", name=f"u{t}")
                        nc.gpsimd.tensor_tensor(u[:], T1st[:], rs[:],
                                                op=_ALU.add)
                        nc.gpsimd.tensor_tensor(u[:], u[:], C99t[:],
                                                op=_ALU.mult)
                        nc.gpsimd.tensor_tensor(T1st[:], u[:], K1t[:],
                                                op=_ALU.add)
                        nc.gpsimd.tensor_tensor(NT1st[:], nTn[:], T1st[:],
                                                op=_ALU.add)
                nc.sync.dma_start(out[:, t0c * 128:(t0c + TC) * 128], acc[:])
                return i_next

            for _rep in range(REPEAT):
                # Software-pipelined emission: iteration c issues chunk c's
                # matmul+evac, then the PREVIOUS chunk's scan (with chunk c's
                # tensor_tensor_scan injected before its last step).
                t0 = 0
                pend = None   # (c, TC, t0, i_all) whose scan is not yet issued
                for c in range(NCH):
                    TC = CHUNKS[c]
                    wt_ev = emit_mm_evac(c, t0, TC)
                    if pend is None:
                        i_all = emit_ttscan(c, TC, wt_ev, None, 0)
                        pend = (c, TC, t0, i_all)
                    else:
                        pc, pTC, pt0, pi = pend
                        i_all = emit_scan(pc, pTC, pt0, pi, (c, TC, wt_ev))
                        pend = (c, TC, t0, i_all)
                    t0 += TC
                pc, pTC, pt0, pi = pend
                emit_scan(pc, pTC, pt0, pi, None)
    nc.compile()
    return nc


def _build(a_mem, a_syn, lr, tgt, wscale=None):
    """Build + compile the per-core Bass graph (same graph on all 8 cores)."""
    return _build_fp8x4(a_mem, a_syn, lr, tgt, wscale)


_CACHE = {}


def _get_nc(a_mem, a_syn, lr, tgt, wscale=None):
    key = (REPEAT, NPLANES, OUT_FP8, tuple(CHUNKS), wscale,
           float(a_mem), float(a_syn), float(lr), float(tgt))
    if key not in _CACHE:
        _CACHE[key] = _build(a_mem, a_syn, lr, tgt, wscale)
    return _CACHE[key]


def kernel(input_spikes, weight, synaptic_strength, threshold,
           tau_mem, tau_syn, target_rate, homeostatic_lr):
    spikes = np.asarray(input_spikes, dtype=np.float32)
    w_eff = (np.asarray(weight, dtype=np.float32)
             * np.asarray(synaptic_strength, dtype=np.float32))
    thr = np.asarray(threshold, dtype=np.float32)
    tau_m = np.float32(tau_mem)
    tau_s = np.float32(tau_syn)
    tgt = np.float32(target_rate)
    lr = np.float32(homeostatic_lr)
    a_mem = np.float32(np.exp(np.float64(np.float32(-DT) / tau_m)))
    a_syn = np.float32(np.exp(np.float64(np.float32(-DT) / tau_s)))

    wscale = float(np.float32(W_SCALE_NUM / max(np.abs(w_eff).max(), 1e-30)))
    nc = _get_nc(a_mem, a_syn, lr, tgt, wscale)

    import ml_dtypes
    # spikes [B,I,T] -> partition-major [i128, (chunk, k, b, tc)]
    sIT = spikes.transpose(1, 0, 2)      # [I, B, T]
    pieces = []
    t0 = 0
    for tc_ in CHUNKS:
        blk = sIT[:, :, t0:t0 + tc_].reshape(KT, 128, B * tc_)
        pieces.append(blk.transpose(1, 0, 2).reshape(128, KT * B * tc_))
        t0 += tc_
    spk8_prep = np.ascontiguousarray(
        np.concatenate(pieces, axis=1)).astype(ml_dtypes.float8_e4m3)

    in_maps = []
    for core in range(NCORES):
        shard = w_eff[:, core * HL:(core + 1) * HL]          # [I, 256]
        wk = shard.reshape(KT, 128, 2, 128).transpose(0, 2, 1, 3)  # [k,ht,i,h]
        r = wk * np.float32(wscale)
        planes = []
        for _p in range(NPLANES):
            q = r.astype(ml_dtypes.float8_e4m3)
            r = r - q.astype(np.float32)
            # [k,ht,i,h] -> [kp,ko,ht,i,h] -> [i,kp,ht,ko,h]
            planes.append(
                q.reshape(KT // 2, 2, 2, 128, 128)
                .transpose(3, 0, 2, 1, 4).reshape(128, KT * 2 * 128))
        wk8 = np.ascontiguousarray(np.stack(planes, axis=1)).reshape(
            128, NPLANES * KT * 2 * 128)
        nt0 = np.ascontiguousarray(
            -thr[core * HL:(core + 1) * HL].reshape(2, 128).T)
        in_maps.append({"nt0": nt0, "wgt8": wk8, "spk8": spk8_prep})

    res = run_bass_kernel_spmd(nc, in_maps, core_ids=list(range(NCORES)),
                               trace=TRACE, **TRACE_KW)
    kernel.last_result = res

    outs = []
    for core in range(NCORES):
        o = res.results[core]["out"]
        # LIF_SC emits 2-scaled spikes; recover {0,1}
        o = (o.astype(np.float32) != 0.0).astype(np.float32)
        o = o.reshape(128, T, 2, B)
        outs.append(o.transpose(3, 2, 0, 1).reshape(B, HL, T))
    return np.ascontiguousarray(np.concatenate(outs, axis=1))
